# revision 15
# baseline (speedup 1.0000x reference)
"""Binarized 3x3 conv (XNOR-style): sign(conv2d(sign(x), sign(w)) + b).

Full-input contract: kernel(x=[32,256,56,56]f32, weight=[256,256,3,3]f32,
bias=[256]f32) -> [32,256,56,56]f32.

Strategy: data-parallel over batch across 8 NeuronCores (4 images/core),
with a 1D Winograd F(2,3) factorization along H that cuts tensor-engine
work 1.5x vs the direct 9-tap formulation (12 instead of 18 row-convs per
2 output rows).

Host prep (exact, integer-valued):
  - t_j = (B^T d)/2 over padded row quadruples d (rows 2b..2b+3 of the
    0-padded image), j=0..3: values in {0,+-0.5,+-1}, stored fp8e4m3.
    Block rows are 57 wide ([Z, r0..r55], one shared zero column), and a
    (g, j) chunk is 7 rows + 2 zero cols = 401, so the tap-tx matmul
    window [401*j + tx : .. + 399] stays inside its own chunk: free dim
    399 vs 406 (-1.7% PE) and no cross-chunk dependencies.
  - w_j = (G sign(w))_j rows: values {+-0.5,+-1,+-1.5}, exact in fp8.
    Slots in consumption order (kg0 j1,j2,j0,j3 | kg1 same | kg1
    -j2,-j3); kg0 needs no negated slots (the direct-PSUM final group is
    kg1).
  - Everything ships in ONE DRAM tensor laid out [w-kg0 | img0-group0 |
    w-kg1 | remaining 15 (img, group) chunks]: DMA cost is ~175ns per
    128-partition descriptor per queue (rows below ~3.2KB are
    latency-bound, not wire-bound) and queues process descriptors FIFO,
    so the whole startup is ordered by consumption deadline and the
    critical first transfer [w-kg0 | g0] is a single 8-descriptor-per-
    queue DMA (~2us) instead of two (~2.9us).
Device per core:
  - Boot: engines boot at ~5.8us; each dma_start costs ~700ns of
    descriptor-gen on its issuing engine, so loads are spread (scalar:
    critical+g2+img1, sync: g1,g3,w-kg1,img2,img3 -- deadline order
    across the shared queue FIFO; gpsimd's SWDGE has a first-use stall
    so it only issues stores). The PE runs NWARM warmup matmuls (gated
    only on the gpsimd wsrc memset) until the first chunk lands ~10.3us.
    CRITICAL: the PE must stay continuously busy from the first warmup
    through the steady stream -- any idle gap during the ~3us clock-ramp
    window locks the DVFS low for the whole run (measured ~10us loss).
  - per (img, kg, group of 7 blocks): 12 fp8 DoubleRow matmuls (contract
    256, free 399) accumulate m_0..m_3 into 4 PSUM banks, emitted in j
    order (1,2,0,3) so staging starts early.
  - evac (ops may read at most one PSUM input; GpSimd none): Scalar
    stages sm1=m1/sm2=m2 to fp16 (exact: m are quarter-integers far
    below fp16's 0.25-step bound of 512, checked by the rel-err gate),
    GpSimd forms s0=sm1+sm2, DVE forms s1=sm1-sm2 plus u0=s0+m0 /
    u1=s1-m3 in fp16 (exact: u values are INTEGERS -- half of the even
    +-1 conv sum -- bounded by 1152 < 2048), Scalar applies
    Sign(u + bias/2) into the (img, kg) staging tile (Sign(0)=0 on HW).
    The last two normal groups shift sm1/s0 onto DVE and s1 onto GpSimd
    so the tail-critical staging chain avoids Scalar's end-of-run sign
    backlog.
  - Output stores are ONE [128, 3136] DMA per (img, kg) issued by GpSimd
    after its 4th group (784B-row per-group stores would cost 4x the
    descriptor time), except the final (img, kg) which stores g0-g2
    early and g3 alone to keep the tail transfer small.
  - The final group accumulates u0/u1 directly in PSUM (18 matmuls using
    negated j2/j3 weight slots) and clamps on DVE -- u0's clamp is
    emitted between the two 9-matmul halves so it overlaps the PE,
    leaving only clamp(u1) + one small store behind the last matmul.
  - All sums are multiples of 0.25 bounded << 2^24 so f32 accumulation is
    exact; sign(conv+b) == sign(conv/2+b/2) by binade-shift exactness.
Output returned as fp8 (+-1/0 exact) and widened to f32 on host.
"""

import numpy as np

import concourse.bacc as bacc
import concourse.mybir as mybir
import concourse.tile as tile
from concourse.bass_utils import run_bass_kernel_spmd

N_CORES = 8
N_PER = 4          # images per core
C = 256            # input channels
K = 256            # output channels
H = W = 56
NBL = 28           # Winograd 2-row blocks per image
NG = 4             # block groups per (img, kg)
GBL = 7            # blocks per group
WROW = 57          # padded row width (1 shared zero col per block row)
FREE = GBL * WROW  # 399 matmul free size
CW = FREE + 2      # 401: (j, i) chunk col width (2 dead pad cols)
CH = 4 * 2 * CW    # 3208: per-(img, group) slab chunk
SLOT = {1: 0, 2: 1, 0: 2, 3: 3}  # j -> slot within a kg block
OW0 = 0            # w kg0 slots (4 x 768)
OG0 = 3072         # img0 group0 chunk
OW1 = OG0 + CH     # 6280: w kg1 + negated slots (6 x 768)
OT = OW1 + 4608    # 10888: remaining (img, group) chunks, deadline order
TOT = OT + 15 * CH  # 59008 columns
NWARM = 28

_cache = {}


def _build(with_bias):
    dt = mybir.dt
    xdt = dt.float8e4
    nc = bacc.Bacc()
    t_d = nc.declare_dram_parameter("tin", [128, TOT], xdt, isOutput=False)
    if with_bias:
        b_d = nc.declare_dram_parameter("bhalf", [128, 2], dt.float32,
                                        isOutput=False)
    o_d = nc.declare_dram_parameter("out", [N_PER, K, H, W], xdt, isOutput=True)

    def chunk_off(n, g):
        return OG0 if (n, g) == (0, 0) else OT + (n * NG + g - 1) * CH

    with tile.TileContext(nc) as tc:
        with (
            tc.tile_pool(name="wpool", bufs=1) as wpool,
            tc.tile_pool(name="upool", bufs=4) as upool,
            tc.tile_pool(name="opool", bufs=2) as o_pool,
            tc.tile_pool(name="psum", bufs=8, space="PSUM") as p_pool,
        ):
            wsrc = wpool.tile([128, 256], xdt)
            big = wpool.tile([128, TOT], xdt)

            # GpSimd: warmup-source memset first (gates PE warmups).
            nc.gpsimd.memset(wsrc[:], 0.0)


            # Each DGE ring (scalar-HWDGE, sync-HWDGE, gpsimd-SWDGE)
            # sustains only ~150B/ns, so the critical first bytes are
            # split across all three (g0 by partition halves, w-kg0 by
            # slot pairs) and every ring's queue is ordered by
            # consumption deadline. Deadlines (us): w-j1j2+g0 ~11,
            # w-j0j3 ~12.2, g1 ~13, g2 ~15.2, g3 ~17.3, w-kg1 ~19.3,
            # img1 ~27.6, img2 ~43, img3 ~60.
            def load(eng, lo, hi, p0=0, p1=128):
                eng.dma_start(big[p0:p1, lo:hi], t_d[p0:p1, lo:hi])

            load(nc.scalar, OG0, OG0 + CH)                 # g0
            load(nc.scalar, chunk_off(0, 2), chunk_off(0, 3))   # g2
            load(nc.scalar, chunk_off(1, 2), chunk_off(2, 0))   # img1 g2,g3
            load(nc.scalar, chunk_off(3, 0), TOT)              # img3
            load(nc.sync, 0, 3072)                         # w-kg0
            load(nc.sync, OW1, OT)                         # w-kg1 + neg
            load(nc.sync, chunk_off(1, 0), chunk_off(1, 2))    # img1 g0,g1
            load(nc.sync, chunk_off(2, 0), chunk_off(3, 0))    # img2
            load(nc.gpsimd, chunk_off(0, 1), chunk_off(0, 2))  # g1
            load(nc.gpsimd, chunk_off(0, 3), chunk_off(1, 0))  # g3
            if with_bias:
                nc.scalar.dma_start(b_sb := wpool.tile([128, 2], dt.float32),
                                    b_d[:])

            # Warm the PE HAM clock gate while the first chunks stream in.
            warm = p_pool.tile([128, 232], dt.float32, tag="ps")
            for _ in range(NWARM):
                nc.tensor.matmul(warm[:], wsrc[:, 0:128], wsrc[:, 0:232],
                                 start=True, stop=True)

            wv0 = big[:, OW0:OW0 + 3072].rearrange(
                "p (s t i k) -> p s t i k", s=4, t=3, i=2)
            wv1 = big[:, OW1:OW1 + 4608].rearrange(
                "p (s t i k) -> p s t i k", s=6, t=3, i=2)

            def wslot(sl, tx):
                return wv0[:, sl, tx] if sl < 4 else wv1[:, sl - 4, tx]

            add, sub = mybir.AluOpType.add, mybir.AluOpType.subtract

            for n in range(N_PER):
                for kg in range(2):
                    osb = None
                    for g in range(NG):
                        if g == 0:
                            osb = o_pool.tile([128, NG * 2 * GBL * W], xdt,
                                              tag="osb")
                        ovg = osb[:, g * 784:(g + 1) * 784].rearrange(
                            "p (b j c) -> p j b c", j=2, c=W)
                        off = chunk_off(n, g)
                        cv = big[:, off:off + CH].rearrange(
                            "p (j i c) -> p j i c", j=4, i=2)
                        # Last two groups: accumulate u0/u1 directly in PSUM
                        # via 18 matmuls each (slots 8/9 hold negated j2/j3),
                        # so both outputs come straight from clamp(psum) with
                        # no staging chain -- +2us of PE, but every engine's
                        # queue fully drains during the ~6us of direct
                        # matmuls, leaving only clamp(u1) + one small store
                        # behind the last matmul.
                        direct = (n == N_PER - 1 and kg == 1 and g >= NG - 2)
                        if direct:
                            if g == NG - 2:
                                # store g0-g1 as soon as their signs land
                                nc.gpsimd.dma_start(
                                    o_d[n, kg * 128:(kg + 1) * 128, 0:28, :],
                                    osb[:, 0:1568])
                            for ji, jws in ((0, ((4, 1), (5, 2), (6, 0))),
                                            (1, ((4, 1), (8, 2), (9, 3)))):
                                pst = p_pool.tile([128, FREE], dt.float32,
                                                  tag="ps", name=f"psF{g}_{ji}")
                                for step, (sl, jt) in enumerate(jws):
                                    for tx in range(3):
                                        nc.tensor.matmul(
                                            pst[:], wslot(sl, tx),
                                            cv[:, jt, :, tx:tx + FREE],
                                            start=(step == 0 and tx == 0),
                                            stop=(step == 2 and tx == 2),
                                            perf_mode=mybir.MatmulPerfMode.DoubleRow,
                                        )
                                pvw = pst[:].rearrange("p (b q) -> p b q",
                                                       b=GBL)[:, :, 0:W]
                                if with_bias:
                                    nc.scalar.sign(ovg[:, ji], pvw,
                                                   bias=b_sb[:, kg:kg + 1])
                                else:
                                    # DVE clamp == sign for integers; emitted
                                    # right after each 9-matmul half so the
                                    # ji=0 clamp overlaps the ji=1 matmuls.
                                    nc.vector.tensor_scalar(
                                        ovg[:, ji], pvw, 1.0, -1.0,
                                        mybir.AluOpType.min,
                                        mybir.AluOpType.max)
                            nc.gpsimd.dma_start(
                                o_d[n, kg * 128:(kg + 1) * 128,
                                    g * 14:(g + 1) * 14, :],
                                osb[:, g * 784:(g + 1) * 784])
                            continue
                        ps = {}
                        # j order (1,2,0,3): m1/m2 finish first so staging
                        # starts early; m0/m3 (read by the late u-ops) are
                        # the last banks the next-next group waits on.
                        for j in (1, 2, 0, 3):
                            ps[j] = p_pool.tile([128, FREE], dt.float32,
                                                tag="ps",
                                                name=f"ps{n}_{kg}_{g}_{j}")
                            sl = kg * 4 + SLOT[j]
                            for tx in range(3):
                                nc.tensor.matmul(
                                    ps[j][:], wslot(sl, tx),
                                    cv[:, j, :, tx:tx + FREE],
                                    start=(tx == 0), stop=(tx == 2),
                                    perf_mode=mybir.MatmulPerfMode.DoubleRow,
                                )
                        # u0 = (m1+m2)+m0, u1 = (m1-m2)-m3, all staged fp16
                        # (exact, see module docstring). The last two normal
                        # groups move sm1/s0 to DVE and s1 to GpSimd so the
                        # tail chain dodges Scalar's sign backlog.
                        tailish = False
                        sm1 = upool.tile([128, FREE], dt.float16, tag="sm1")
                        sm2 = upool.tile([128, FREE], dt.float16, tag="sm2")
                        s0 = upool.tile([128, FREE], dt.float16, tag="s0")
                        s1 = upool.tile([128, FREE], dt.float16, tag="s1")
                        u01 = upool.tile([128, 2 * FREE], dt.float16, tag="u01")
                        if tailish:
                            nc.vector.tensor_scalar_add(sm1[:], ps[1][:], 0.0)
                            nc.scalar.copy(sm2[:], ps[2][:])
                            nc.vector.tensor_tensor(s0[:], sm1[:], sm2[:], add)
                            nc.gpsimd.tensor_tensor(s1[:], sm1[:], sm2[:], sub)
                        else:
                            nc.scalar.copy(sm1[:], ps[1][:])
                            nc.scalar.copy(sm2[:], ps[2][:])
                            nc.gpsimd.tensor_tensor(s0[:], sm1[:], sm2[:], add)
                            nc.vector.tensor_tensor(s1[:], sm1[:], sm2[:], sub)
                        nc.vector.tensor_tensor(u01[:, 0:FREE], s0[:], ps[0][:], add)
                        nc.vector.tensor_tensor(u01[:, FREE:], s1[:], ps[3][:], sub)
                        uv = u01[:].rearrange("p (j b q) -> p j b q", j=2,
                                              b=GBL)[:, :, :, 0:W]
                        if with_bias:
                            nc.scalar.sign(ovg, uv, bias=b_sb[:, kg:kg + 1])
                        else:
                            nc.scalar.sign(ovg, uv)
                        if g == NG - 1:
                            # one big store per (img, kg): 3136B descriptors
                            nc.gpsimd.dma_start(
                                o_d[n, kg * 128:(kg + 1) * 128, :, :],
                                osb[:, 0:3136])

    nc.finalize()
    return nc


_T_LUT = np.array([0xB8, 0xB0, 0x00, 0x30, 0x38], np.uint8)       # v/2, v=-2..2
_W_LUT = np.array([0xBC, 0xB8, 0xB0, 0x00, 0x30, 0x38, 0x3C], np.uint8)


def _prep_inputs(x, wsgn):
    """x [32,256,56,56] f32 + wsgn [128, 7680] -> per-core [8][128, TOT]."""
    s = np.sign(x).astype(np.int8)
    xp = np.zeros((N_CORES * N_PER, C, 58, 58), np.int8)
    xp[:, :, 1:57, 1:57] = s
    d0 = xp[:, :, 0:56:2, :]
    d1 = xp[:, :, 1:57:2, :]
    d2 = xp[:, :, 2:58:2, :]
    d3 = xp[:, :, 3:58:2, :]
    t = np.empty((N_CORES * N_PER, C, 4, NBL, 58), np.int8)
    t[:, :, 0] = d0 - d2
    t[:, :, 1] = d1 + d2
    t[:, :, 2] = d2 - d1
    t[:, :, 3] = d1 - d3
    tb = _T_LUT[t[..., 0:WROW] + 2]                       # [32,256,4,28,57]
    v = tb.reshape(N_CORES, N_PER, 2, 128, 4, NG, GBL, WROW)
    v = v.transpose(0, 3, 1, 5, 4, 2, 6, 7)   # [core,p,n,g,j,i,b,w]
    ch = np.zeros((N_CORES, 128, N_PER, NG, 4, 2, CW), np.uint8)
    ch[..., :FREE] = v.reshape(N_CORES, 128, N_PER, NG, 4, 2, FREE)
    ch = ch.reshape(N_CORES, 128, 16, CH)
    out = np.empty((N_CORES, 128, TOT), np.uint8)
    out[:, :, OW0:OW0 + 3072] = wsgn[:, 0:3072]
    out[:, :, OG0:OG0 + CH] = ch[:, :, 0]
    out[:, :, OW1:OW1 + 4608] = wsgn[:, 3072:7680]
    out[:, :, OT:] = ch[:, :, 1:].reshape(N_CORES, 128, 15 * CH)
    return out.view(mybir.dt.np(mybir.dt.float8e4))


def _prep_weights(weight):
    s = np.sign(weight.astype(np.float32)).astype(np.int8)  # [k, c, ty, tx]
    w0, w1, w2 = s[:, :, 0, :], s[:, :, 1, :], s[:, :, 2, :]
    g = np.empty((6, K, C, 3), np.int8)  # 2*(G w)_j (+ negated j2/j3)
    g[0] = 2 * w0
    g[1] = w0 + w1 + w2
    g[2] = w0 - w1 + w2
    g[3] = 2 * w2
    g[4] = -g[2]
    g[5] = -g[3]
    gb = _W_LUT[g + 3]
    full = gb.reshape(6, 2, 128, 2, 128, 3)      # [j, kg, kk, i, p, tx]
    order = [(1, 0), (2, 0), (0, 0), (3, 0),
             (1, 1), (2, 1), (0, 1), (3, 1), (4, 1), (5, 1)]
    sl = np.stack([full[j, kg] for (j, kg) in order])  # [s, kk, i, p, tx]
    arr = sl.transpose(3, 0, 4, 2, 1)            # [p, s, tx, i, kk]
    return np.ascontiguousarray(arr).reshape(128, 10 * 768)


def kernel(x, weight, bias, _profile=False, _trace_kwargs=None):
    x = np.asarray(x, dtype=np.float32)
    weight = np.asarray(weight, dtype=np.float32)
    bias = np.asarray(bias, dtype=np.float32)
    assert x.shape == (N_CORES * N_PER, C, H, W), x.shape
    assert weight.shape == (K, C, 3, 3), weight.shape
    assert bias.shape == (K,), bias.shape
    with_bias = bool(np.any(bias != 0.0))

    if with_bias not in _cache:
        _cache[with_bias] = _build(with_bias)
    nc = _cache[with_bias]

    wsgn = _prep_weights(weight)
    tin = _prep_inputs(x, wsgn)
    in_maps = []
    for c in range(N_CORES):
        m = {"tin": np.ascontiguousarray(tin[c])}
        if with_bias:
            m["bhalf"] = np.ascontiguousarray(
                (bias.reshape(2, 128).T * 0.5).astype(np.float32))
        in_maps.append(m)

    res = run_bass_kernel_spmd(
        nc, in_maps, core_ids=list(range(N_CORES)),
        trace=_profile, **(_trace_kwargs or {}),
    )
    out = np.concatenate([res.results[c]["out"] for c in range(N_CORES)],
                         axis=0).astype(np.float32)
    if _profile:
        kernel.last_exec_ns = res.exec_time_ns
        kernel.last_results = res
    return out


# revision 16
# speedup vs baseline: 1.0272x; 1.0272x over previous
"""Binarized 3x3 conv (XNOR-style): sign(conv2d(sign(x), sign(w)) + b).

Full-input contract: kernel(x=[32,256,56,56]f32, weight=[256,256,3,3]f32,
bias=[256]f32) -> [32,256,56,56]f32.

Strategy: data-parallel over batch across 8 NeuronCores (4 images/core),
with a 1D Winograd F(2,3) factorization along H that cuts tensor-engine
work 1.5x vs the direct 9-tap formulation (12 instead of 18 row-convs per
2 output rows).

Host prep (exact, integer-valued):
  - t_j = (B^T d)/2 over padded row quadruples d (rows 2b..2b+3 of the
    0-padded image), j=0..3: values in {0,+-0.5,+-1}, stored fp8e4m3.
    Block rows are 57 wide ([Z, r0..r55], one shared zero column), and a
    (g, j) chunk is 7 rows + 2 zero cols = 401, so the tap-tx matmul
    window [401*j + tx : .. + 399] stays inside its own chunk: free dim
    399 vs 406 (-1.7% PE) and no cross-chunk dependencies.
  - w_j = (G sign(w))_j rows: values {+-0.5,+-1,+-1.5}, exact in fp8.
    Slots in consumption order (kg0 j1,j2,j0,j3 | kg1 same | kg1
    -j2,-j3); kg0 needs no negated slots (the direct-PSUM final group is
    kg1).
  - Everything ships in ONE DRAM tensor laid out [w-kg0 | img0-group0 |
    w-kg1 | remaining 15 (img, group) chunks]: DMA cost is ~175ns per
    128-partition descriptor per queue (rows below ~3.2KB are
    latency-bound, not wire-bound) and queues process descriptors FIFO,
    so the whole startup is ordered by consumption deadline and the
    critical first transfer [w-kg0 | g0] is a single 8-descriptor-per-
    queue DMA (~2us) instead of two (~2.9us).
Device per core:
  - Boot: engines boot at ~5.8us; each dma_start costs ~700ns of
    descriptor-gen on its issuing engine, so loads are spread (scalar:
    critical+g2+img1, sync: g1,g3,w-kg1,img2,img3 -- deadline order
    across the shared queue FIFO; gpsimd's SWDGE has a first-use stall
    so it only issues stores). The PE runs NWARM warmup matmuls (gated
    only on the gpsimd wsrc memset) until the first chunk lands ~10.3us.
    CRITICAL: the PE must stay continuously busy from the first warmup
    through the steady stream -- any idle gap during the ~3us clock-ramp
    window locks the DVFS low for the whole run (measured ~10us loss).
  - per (img, kg, group of 7 blocks): 12 fp8 DoubleRow matmuls (contract
    256, free 399) accumulate m_0..m_3 into 4 PSUM banks, emitted in j
    order (1,2,0,3) so staging starts early.
  - evac (ops may read at most one PSUM input; GpSimd none): Scalar
    stages sm1=m1/sm2=m2 to fp16 (exact: m are quarter-integers far
    below fp16's 0.25-step bound of 512, checked by the rel-err gate),
    GpSimd forms s0=sm1+sm2, DVE forms s1=sm1-sm2 plus u0=s0+m0 /
    u1=s1-m3 in fp16 (exact: u values are INTEGERS -- half of the even
    +-1 conv sum -- bounded by 1152 < 2048), Scalar applies
    Sign(u + bias/2) into the (img, kg) staging tile (Sign(0)=0 on HW).
    The last two normal groups shift sm1/s0 onto DVE and s1 onto GpSimd
    so the tail-critical staging chain avoids Scalar's end-of-run sign
    backlog.
  - Output stores are ONE [128, 3136] DMA per (img, kg) issued by GpSimd
    after its 4th group (784B-row per-group stores would cost 4x the
    descriptor time), except the final (img, kg) which stores g0-g2
    early and g3 alone to keep the tail transfer small.
  - The final group accumulates u0/u1 directly in PSUM (18 matmuls using
    negated j2/j3 weight slots) and clamps on DVE -- u0's clamp is
    emitted between the two 9-matmul halves so it overlaps the PE,
    leaving only clamp(u1) + one small store behind the last matmul.
  - All sums are multiples of 0.25 bounded << 2^24 so f32 accumulation is
    exact; sign(conv+b) == sign(conv/2+b/2) by binade-shift exactness.
Output returned as fp8 (+-1/0 exact) and widened to f32 on host.
"""

import numpy as np

import concourse.bacc as bacc
import concourse.mybir as mybir
import concourse.tile as tile
from concourse.bass_utils import run_bass_kernel_spmd

N_CORES = 8
N_PER = 4          # images per core
C = 256            # input channels
K = 256            # output channels
H = W = 56
NBL = 28           # Winograd 2-row blocks per image
NG = 4             # block groups per (img, kg)
GBL = 7            # blocks per group
WROW = 57          # padded row width (1 shared zero col per block row)
FREE = GBL * WROW  # 399 matmul free size
CW = FREE + 2      # 401: (j, i) chunk col width (2 dead pad cols)
CH = 4 * 2 * CW    # 3208: per-(img, group) slab chunk
SLOT = {1: 0, 2: 1, 0: 2, 3: 3}  # j -> slot within a kg block
OW0 = 0            # w kg0 slots (4 x 768)
OG0 = 3072         # img0 group0 chunk
OW1 = OG0 + CH     # 6280: w kg1 + negated slots (6 x 768)
OT = OW1 + 4608    # 10888: remaining (img, group) chunks, deadline order
TOT = OT + 15 * CH  # 59008 columns
NWARM = 24

_cache = {}


def _build(with_bias):
    dt = mybir.dt
    xdt = dt.float8e4
    nc = bacc.Bacc()
    t_d = nc.declare_dram_parameter("tin", [128, TOT], xdt, isOutput=False)
    if with_bias:
        b_d = nc.declare_dram_parameter("bhalf", [128, 2], dt.float32,
                                        isOutput=False)
    o_d = nc.declare_dram_parameter("out", [N_PER, K, H, W], xdt, isOutput=True)

    def chunk_off(n, g):
        return OG0 if (n, g) == (0, 0) else OT + (n * NG + g - 1) * CH

    with tile.TileContext(nc) as tc:
        with (
            tc.tile_pool(name="wpool", bufs=1) as wpool,
            tc.tile_pool(name="upool", bufs=4) as upool,
            tc.tile_pool(name="opool", bufs=2) as o_pool,
            tc.tile_pool(name="psum", bufs=8, space="PSUM") as p_pool,
        ):
            wsrc = wpool.tile([128, 256], xdt)
            big = wpool.tile([128, TOT], xdt)

            # GpSimd: warmup-source memset first (gates PE warmups).
            nc.gpsimd.memset(wsrc[:], 0.0)


            # Each DGE ring (scalar-HWDGE, sync-HWDGE, gpsimd-SWDGE)
            # sustains only ~150B/ns, so the critical first bytes are
            # split across all three (g0 by partition halves, w-kg0 by
            # slot pairs) and every ring's queue is ordered by
            # consumption deadline. Deadlines (us): w-j1j2+g0 ~11,
            # w-j0j3 ~12.2, g1 ~13, g2 ~15.2, g3 ~17.3, w-kg1 ~19.3,
            # img1 ~27.6, img2 ~43, img3 ~60.
            def load(eng, lo, hi, p0=0, p1=128):
                eng.dma_start(big[p0:p1, lo:hi], t_d[p0:p1, lo:hi])

            load(nc.scalar, OG0, OG0 + CH)                 # g0
            load(nc.scalar, chunk_off(0, 2), chunk_off(0, 3))   # g2
            load(nc.scalar, OW1, OT)                       # w-kg1 + neg
            load(nc.scalar, chunk_off(1, 2), chunk_off(2, 0))   # img1 g2,g3
            load(nc.scalar, chunk_off(3, 0), TOT)              # img3
            load(nc.sync, 0, 3072)                         # w-kg0
            load(nc.sync, chunk_off(0, 3), chunk_off(1, 0))    # g3
            load(nc.sync, chunk_off(1, 0), chunk_off(1, 2))    # img1 g0,g1
            load(nc.sync, chunk_off(2, 0), chunk_off(3, 0))    # img2
            load(nc.gpsimd, chunk_off(0, 1), chunk_off(0, 2))  # g1
            if with_bias:
                nc.scalar.dma_start(b_sb := wpool.tile([128, 2], dt.float32),
                                    b_d[:])

            # Warm the PE HAM clock gate while the first chunks stream in.
            warm = p_pool.tile([128, 232], dt.float32, tag="ps")
            for _ in range(NWARM):
                nc.tensor.matmul(warm[:], wsrc[:, 0:128], wsrc[:, 0:232],
                                 start=True, stop=True)

            wv0 = big[:, OW0:OW0 + 3072].rearrange(
                "p (s t i k) -> p s t i k", s=4, t=3, i=2)
            wv1 = big[:, OW1:OW1 + 4608].rearrange(
                "p (s t i k) -> p s t i k", s=6, t=3, i=2)

            def wslot(sl, tx):
                return wv0[:, sl, tx] if sl < 4 else wv1[:, sl - 4, tx]

            add, sub = mybir.AluOpType.add, mybir.AluOpType.subtract

            for n in range(N_PER):
                for kg in range(2):
                    osb = None
                    for g in range(NG):
                        if g == 0:
                            osb = o_pool.tile([128, NG * 2 * GBL * W], xdt,
                                              tag="osb")
                        ovg = osb[:, g * 784:(g + 1) * 784].rearrange(
                            "p (b j c) -> p j b c", j=2, c=W)
                        off = chunk_off(n, g)
                        cv = big[:, off:off + CH].rearrange(
                            "p (j i c) -> p j i c", j=4, i=2)
                        # Last two groups: accumulate u0/u1 directly in PSUM
                        # via 18 matmuls each (slots 8/9 hold negated j2/j3),
                        # so both outputs come straight from clamp(psum) with
                        # no staging chain -- +2us of PE, but every engine's
                        # queue fully drains during the ~6us of direct
                        # matmuls, leaving only clamp(u1) + one small store
                        # behind the last matmul.
                        direct = (n == N_PER - 1 and kg == 1 and g >= NG - 2)
                        if direct:
                            if g == NG - 2:
                                # store g0-g1 as soon as their signs land
                                nc.gpsimd.dma_start(
                                    o_d[n, kg * 128:(kg + 1) * 128, 0:28, :],
                                    osb[:, 0:1568])
                            for ji, jws in ((0, ((4, 1), (5, 2), (6, 0))),
                                            (1, ((4, 1), (8, 2), (9, 3)))):
                                pst = p_pool.tile([128, FREE], dt.float32,
                                                  tag="ps", name=f"psF{g}_{ji}")
                                for step, (sl, jt) in enumerate(jws):
                                    for tx in range(3):
                                        nc.tensor.matmul(
                                            pst[:], wslot(sl, tx),
                                            cv[:, jt, :, tx:tx + FREE],
                                            start=(step == 0 and tx == 0),
                                            stop=(step == 2 and tx == 2),
                                            perf_mode=mybir.MatmulPerfMode.DoubleRow,
                                        )
                                pvw = pst[:].rearrange("p (b q) -> p b q",
                                                       b=GBL)[:, :, 0:W]
                                if with_bias:
                                    nc.scalar.sign(ovg[:, ji], pvw,
                                                   bias=b_sb[:, kg:kg + 1])
                                else:
                                    # DVE clamp == sign for integers; emitted
                                    # right after each 9-matmul half so the
                                    # ji=0 clamp overlaps the ji=1 matmuls.
                                    nc.vector.tensor_scalar(
                                        ovg[:, ji], pvw, 1.0, -1.0,
                                        mybir.AluOpType.min,
                                        mybir.AluOpType.max)
                            nc.gpsimd.dma_start(
                                o_d[n, kg * 128:(kg + 1) * 128,
                                    g * 14:(g + 1) * 14, :],
                                osb[:, g * 784:(g + 1) * 784])
                            continue
                        ps = {}
                        # j order (1,2,0,3): m1/m2 finish first so staging
                        # starts early; m0/m3 (read by the late u-ops) are
                        # the last banks the next-next group waits on.
                        for j in (1, 2, 0, 3):
                            ps[j] = p_pool.tile([128, FREE], dt.float32,
                                                tag="ps",
                                                name=f"ps{n}_{kg}_{g}_{j}")
                            sl = kg * 4 + SLOT[j]
                            for tx in range(3):
                                nc.tensor.matmul(
                                    ps[j][:], wslot(sl, tx),
                                    cv[:, j, :, tx:tx + FREE],
                                    start=(tx == 0), stop=(tx == 2),
                                    perf_mode=mybir.MatmulPerfMode.DoubleRow,
                                )
                        # u0 = (m1+m2)+m0, u1 = (m1-m2)-m3, all staged fp16
                        # (exact, see module docstring). The last two normal
                        # groups move sm1/s0 to DVE and s1 to GpSimd so the
                        # tail chain dodges Scalar's sign backlog.
                        tailish = False
                        sm1 = upool.tile([128, FREE], dt.float16, tag="sm1")
                        sm2 = upool.tile([128, FREE], dt.float16, tag="sm2")
                        s0 = upool.tile([128, FREE], dt.float16, tag="s0")
                        s1 = upool.tile([128, FREE], dt.float16, tag="s1")
                        u01 = upool.tile([128, 2 * FREE], dt.float16, tag="u01")
                        if tailish:
                            nc.vector.tensor_scalar_add(sm1[:], ps[1][:], 0.0)
                            nc.scalar.copy(sm2[:], ps[2][:])
                            nc.vector.tensor_tensor(s0[:], sm1[:], sm2[:], add)
                            nc.gpsimd.tensor_tensor(s1[:], sm1[:], sm2[:], sub)
                        else:
                            nc.scalar.copy(sm1[:], ps[1][:])
                            nc.scalar.copy(sm2[:], ps[2][:])
                            nc.gpsimd.tensor_tensor(s0[:], sm1[:], sm2[:], add)
                            nc.vector.tensor_tensor(s1[:], sm1[:], sm2[:], sub)
                        nc.vector.tensor_tensor(u01[:, 0:FREE], s0[:], ps[0][:], add)
                        nc.vector.tensor_tensor(u01[:, FREE:], s1[:], ps[3][:], sub)
                        uv = u01[:].rearrange("p (j b q) -> p j b q", j=2,
                                              b=GBL)[:, :, :, 0:W]
                        if with_bias:
                            nc.scalar.sign(ovg, uv, bias=b_sb[:, kg:kg + 1])
                        else:
                            nc.scalar.sign(ovg, uv)
                        if g == NG - 1:
                            # one big store per (img, kg): 3136B descriptors
                            nc.gpsimd.dma_start(
                                o_d[n, kg * 128:(kg + 1) * 128, :, :],
                                osb[:, 0:3136])

    nc.finalize()
    return nc


_T_LUT = np.array([0xB8, 0xB0, 0x00, 0x30, 0x38], np.uint8)       # v/2, v=-2..2
_W_LUT = np.array([0xBC, 0xB8, 0xB0, 0x00, 0x30, 0x38, 0x3C], np.uint8)


def _prep_inputs(x, wsgn):
    """x [32,256,56,56] f32 + wsgn [128, 7680] -> per-core [8][128, TOT]."""
    s = np.sign(x).astype(np.int8)
    xp = np.zeros((N_CORES * N_PER, C, 58, 58), np.int8)
    xp[:, :, 1:57, 1:57] = s
    d0 = xp[:, :, 0:56:2, :]
    d1 = xp[:, :, 1:57:2, :]
    d2 = xp[:, :, 2:58:2, :]
    d3 = xp[:, :, 3:58:2, :]
    t = np.empty((N_CORES * N_PER, C, 4, NBL, 58), np.int8)
    t[:, :, 0] = d0 - d2
    t[:, :, 1] = d1 + d2
    t[:, :, 2] = d2 - d1
    t[:, :, 3] = d1 - d3
    tb = _T_LUT[t[..., 0:WROW] + 2]                       # [32,256,4,28,57]
    v = tb.reshape(N_CORES, N_PER, 2, 128, 4, NG, GBL, WROW)
    v = v.transpose(0, 3, 1, 5, 4, 2, 6, 7)   # [core,p,n,g,j,i,b,w]
    ch = np.zeros((N_CORES, 128, N_PER, NG, 4, 2, CW), np.uint8)
    ch[..., :FREE] = v.reshape(N_CORES, 128, N_PER, NG, 4, 2, FREE)
    ch = ch.reshape(N_CORES, 128, 16, CH)
    out = np.empty((N_CORES, 128, TOT), np.uint8)
    out[:, :, OW0:OW0 + 3072] = wsgn[:, 0:3072]
    out[:, :, OG0:OG0 + CH] = ch[:, :, 0]
    out[:, :, OW1:OW1 + 4608] = wsgn[:, 3072:7680]
    out[:, :, OT:] = ch[:, :, 1:].reshape(N_CORES, 128, 15 * CH)
    return out.view(mybir.dt.np(mybir.dt.float8e4))


def _prep_weights(weight):
    s = np.sign(weight.astype(np.float32)).astype(np.int8)  # [k, c, ty, tx]
    w0, w1, w2 = s[:, :, 0, :], s[:, :, 1, :], s[:, :, 2, :]
    g = np.empty((6, K, C, 3), np.int8)  # 2*(G w)_j (+ negated j2/j3)
    g[0] = 2 * w0
    g[1] = w0 + w1 + w2
    g[2] = w0 - w1 + w2
    g[3] = 2 * w2
    g[4] = -g[2]
    g[5] = -g[3]
    gb = _W_LUT[g + 3]
    full = gb.reshape(6, 2, 128, 2, 128, 3)      # [j, kg, kk, i, p, tx]
    order = [(1, 0), (2, 0), (0, 0), (3, 0),
             (1, 1), (2, 1), (0, 1), (3, 1), (4, 1), (5, 1)]
    sl = np.stack([full[j, kg] for (j, kg) in order])  # [s, kk, i, p, tx]
    arr = sl.transpose(3, 0, 4, 2, 1)            # [p, s, tx, i, kk]
    return np.ascontiguousarray(arr).reshape(128, 10 * 768)


def kernel(x, weight, bias, _profile=False, _trace_kwargs=None):
    x = np.asarray(x, dtype=np.float32)
    weight = np.asarray(weight, dtype=np.float32)
    bias = np.asarray(bias, dtype=np.float32)
    assert x.shape == (N_CORES * N_PER, C, H, W), x.shape
    assert weight.shape == (K, C, 3, 3), weight.shape
    assert bias.shape == (K,), bias.shape
    with_bias = bool(np.any(bias != 0.0))

    if with_bias not in _cache:
        _cache[with_bias] = _build(with_bias)
    nc = _cache[with_bias]

    wsgn = _prep_weights(weight)
    tin = _prep_inputs(x, wsgn)
    in_maps = []
    for c in range(N_CORES):
        m = {"tin": np.ascontiguousarray(tin[c])}
        if with_bias:
            m["bhalf"] = np.ascontiguousarray(
                (bias.reshape(2, 128).T * 0.5).astype(np.float32))
        in_maps.append(m)

    res = run_bass_kernel_spmd(
        nc, in_maps, core_ids=list(range(N_CORES)),
        trace=_profile, **(_trace_kwargs or {}),
    )
    out = np.concatenate([res.results[c]["out"] for c in range(N_CORES)],
                         axis=0).astype(np.float32)
    if _profile:
        kernel.last_exec_ns = res.exec_time_ns
        kernel.last_results = res
    return out


# revision 18
# speedup vs baseline: 1.0272x; 1.0001x over previous
"""Binarized 3x3 conv (XNOR-style): sign(conv2d(sign(x), sign(w)) + b).

Full-input contract: kernel(x=[32,256,56,56]f32, weight=[256,256,3,3]f32,
bias=[256]f32) -> [32,256,56,56]f32.

Strategy: data-parallel over batch across 8 NeuronCores (4 images/core),
with a 1D Winograd F(2,3) factorization along H that cuts tensor-engine
work 1.5x vs the direct 9-tap formulation (12 instead of 18 row-convs per
2 output rows).

Host prep (exact, integer-valued):
  - t_j = (B^T d)/2 over padded row quadruples d (rows 2b..2b+3 of the
    0-padded image), j=0..3: values in {0,+-0.5,+-1}, stored fp8e4m3.
    Block rows are 57 wide ([Z, r0..r55], one shared zero column), and a
    (g, j) chunk is 7 rows + 2 zero cols = 401, so the tap-tx matmul
    window [401*j + tx : .. + 399] stays inside its own chunk: free dim
    399 vs 406 (-1.7% PE) and no cross-chunk dependencies.
  - w_j = (G sign(w))_j rows: values {+-0.5,+-1,+-1.5}, exact in fp8.
    Slots in consumption order (kg0 j1,j2,j0,j3 | kg1 same | kg1
    -j2,-j3); kg0 needs no negated slots (the direct-PSUM final group is
    kg1).
  - Everything ships in ONE DRAM tensor laid out [w-kg0 | img0-group0 |
    w-kg1 | remaining 15 (img, group) chunks]: DMA cost is ~175ns per
    128-partition descriptor per queue (rows below ~3.2KB are
    latency-bound, not wire-bound) and queues process descriptors FIFO,
    so the whole startup is ordered by consumption deadline and the
    critical first transfer [w-kg0 | g0] is a single 8-descriptor-per-
    queue DMA (~2us) instead of two (~2.9us).
Device per core:
  - Boot: engines boot at ~5.8us; each dma_start costs ~700ns of
    descriptor-gen on its issuing engine, and each DGE ring (scalar /
    sync / gpsimd) delivers a 128-descriptor load in ~3.3us, so the
    critical stream {w-kg0, g0..g3, w-kg1} is spread across all three
    rings in consumption-deadline order (kg1 reuses the same t chunks,
    so after w-kg1 lands the PE has 16.6us of work queued). The PE runs
    NWARM warmup matmuls (gated only on the gpsimd wsrc memset) until
    the first chunks land ~11.6us.
    CRITICAL: the PE must stay continuously busy from the first warmup
    through the steady stream -- any idle gap during the ~3us clock-ramp
    window locks the DVFS low for the whole run (measured ~10us loss).
  - per (img, kg, group of 7 blocks): 12 fp8 DoubleRow matmuls (contract
    256, free 399) accumulate m_0..m_3 into 4 PSUM banks, emitted in j
    order (1,2,0,3) so staging starts early.
  - evac (ops may read at most one PSUM input; GpSimd none): Scalar
    stages sm1=m1/sm2=m2 to fp16 (exact: m are quarter-integers far
    below fp16's 0.25-step bound of 512, checked by the rel-err gate),
    GpSimd forms s0=sm1+sm2, DVE forms s1=sm1-sm2 plus u0=s0+m0 /
    u1=s1-m3 in fp16 (exact: u values are INTEGERS -- half of the even
    +-1 conv sum -- bounded by 1152 < 2048), Scalar applies
    Sign(u + bias/2) into the (img, kg) staging tile (Sign(0)=0 on HW).
  - Output stores are ONE [128, 3136] DMA per (img, kg) issued by GpSimd
    after its 4th group (784B-row per-group stores would cost 4x the
    descriptor time), except the final (img, kg) which stores g0-g1
    when their signs land and g2/g3 per-group so the tail transfer is
    one small [128, 784] DMA.
  - The last TWO groups accumulate u0/u1 directly in PSUM (18 matmuls
    each, using negated j2/j3 weight slots) and clamp on DVE: every
    engine's queue fully drains during those ~6us of matmuls, leaving
    only clamp(u1) + one small store behind the last matmul.
  - All sums are multiples of 0.25 bounded << 2^24 so f32 accumulation is
    exact; sign(conv+b) == sign(conv/2+b/2) by binade-shift exactness.
Output returned as fp8 (+-1/0 exact) and widened to f32 on host.
"""

import numpy as np

import concourse.bacc as bacc
import concourse.mybir as mybir
import concourse.tile as tile
from concourse.bass_utils import run_bass_kernel_spmd

N_CORES = 8
N_PER = 4          # images per core
C = 256            # input channels
K = 256            # output channels
H = W = 56
NBL = 28           # Winograd 2-row blocks per image
NG = 4             # block groups per (img, kg)
GBL = 7            # blocks per group
WROW = 57          # padded row width (1 shared zero col per block row)
FREE = GBL * WROW  # 399 matmul free size
CW = FREE + 2      # 401: (j, i) chunk col width (2 dead pad cols)
CH = 4 * 2 * CW    # 3208: per-(img, group) slab chunk
SLOT = {1: 0, 2: 1, 0: 2, 3: 3}  # j -> slot within a kg block
OW0 = 0            # w kg0 slots (4 x 768)
OG0 = 3072         # img0 group0 chunk
OW1 = OG0 + CH     # 6280: w kg1 + negated slots (6 x 768)
OT = OW1 + 4608    # 10888: remaining (img, group) chunks, deadline order
TOT = OT + 15 * CH  # 59008 columns
NWARM = 26

_cache = {}


def _build(with_bias):
    dt = mybir.dt
    xdt = dt.float8e4
    nc = bacc.Bacc()
    t_d = nc.declare_dram_parameter("tin", [128, TOT], xdt, isOutput=False)
    if with_bias:
        b_d = nc.declare_dram_parameter("bhalf", [128, 2], dt.float32,
                                        isOutput=False)
    o_d = nc.declare_dram_parameter("out", [N_PER, K, H, W], xdt, isOutput=True)

    def chunk_off(n, g):
        return OG0 if (n, g) == (0, 0) else OT + (n * NG + g - 1) * CH

    with tile.TileContext(nc) as tc:
        with (
            tc.tile_pool(name="wpool", bufs=1) as wpool,
            tc.tile_pool(name="upool", bufs=4) as upool,
            tc.tile_pool(name="opool", bufs=2) as o_pool,
            tc.tile_pool(name="psum", bufs=8, space="PSUM") as p_pool,
        ):
            wsrc = wpool.tile([128, 256], xdt)
            big = wpool.tile([128, TOT], xdt)

            # GpSimd: warmup-source memset first (gates PE warmups).
            nc.gpsimd.memset(wsrc[:], 0.0)


            # Each DGE ring (scalar-HWDGE, sync-HWDGE, gpsimd-SWDGE)
            # delivers a 128-descriptor load in ~3.3us, serially per
            # ring, so the startup stream is spread across rings in
            # consumption-deadline order. Deadlines (us): w-kg0+g0
            # ~11.6 (gates mm1), g1 ~14, g2 ~16.3, g3 ~18.4,
            # w-kg1 ~20.5, img1 ~28.8, img2 ~45, img3 ~62.
            def load(eng, lo, hi, p0=0, p1=128):
                eng.dma_start(big[p0:p1, lo:hi], t_d[p0:p1, lo:hi])

            load(nc.scalar, OG0, OG0 + CH)                 # g0
            load(nc.scalar, chunk_off(0, 3), chunk_off(1, 0))   # g3
            load(nc.scalar, OW1, OT)                       # w-kg1 + neg
            load(nc.scalar, chunk_off(1, 2), chunk_off(2, 0))   # img1 g2,g3
            load(nc.scalar, chunk_off(3, 0), TOT)              # img3
            load(nc.sync, 0, 3072)                         # w-kg0
            load(nc.sync, chunk_off(0, 2), chunk_off(0, 3))    # g2
            load(nc.sync, chunk_off(1, 0), chunk_off(1, 2))    # img1 g0,g1
            load(nc.sync, chunk_off(2, 0), chunk_off(3, 0))    # img2
            load(nc.gpsimd, chunk_off(0, 1), chunk_off(0, 2))  # g1
            if with_bias:
                nc.scalar.dma_start(b_sb := wpool.tile([128, 2], dt.float32),
                                    b_d[:])

            # Warm the PE HAM clock gate while the first chunks stream in.
            warm = p_pool.tile([128, 232], dt.float32, tag="ps")
            for _ in range(NWARM):
                nc.tensor.matmul(warm[:], wsrc[:, 0:128], wsrc[:, 0:232],
                                 start=True, stop=True)

            wv0 = big[:, OW0:OW0 + 3072].rearrange(
                "p (s t i k) -> p s t i k", s=4, t=3, i=2)
            wv1 = big[:, OW1:OW1 + 4608].rearrange(
                "p (s t i k) -> p s t i k", s=6, t=3, i=2)

            def wslot(sl, tx):
                return wv0[:, sl, tx] if sl < 4 else wv1[:, sl - 4, tx]

            add, sub = mybir.AluOpType.add, mybir.AluOpType.subtract

            for n in range(N_PER):
                for kg in range(2):
                    osb = None
                    for g in range(NG):
                        if g == 0:
                            osb = o_pool.tile([128, NG * 2 * GBL * W], xdt,
                                              tag="osb")
                        ovg = osb[:, g * 784:(g + 1) * 784].rearrange(
                            "p (b j c) -> p j b c", j=2, c=W)
                        off = chunk_off(n, g)
                        cv = big[:, off:off + CH].rearrange(
                            "p (j i c) -> p j i c", j=4, i=2)
                        # Last two groups: accumulate u0/u1 directly in PSUM
                        # via 18 matmuls each (slots 8/9 hold negated j2/j3),
                        # so both outputs come straight from clamp(psum) with
                        # no staging chain -- +2us of PE, but every engine's
                        # queue fully drains during the ~6us of direct
                        # matmuls, leaving only clamp(u1) + one small store
                        # behind the last matmul.
                        direct = (n == N_PER - 1 and kg == 1 and g >= NG - 2)
                        if direct:
                            if g == NG - 2:
                                # store g0-g1 as soon as their signs land
                                nc.gpsimd.dma_start(
                                    o_d[n, kg * 128:(kg + 1) * 128, 0:28, :],
                                    osb[:, 0:1568])
                            for ji, jws in ((0, ((4, 1), (5, 2), (6, 0))),
                                            (1, ((4, 1), (8, 2), (9, 3)))):
                                pst = p_pool.tile([128, FREE], dt.float32,
                                                  tag="ps", name=f"psF{g}_{ji}")
                                for step, (sl, jt) in enumerate(jws):
                                    for tx in range(3):
                                        nc.tensor.matmul(
                                            pst[:], wslot(sl, tx),
                                            cv[:, jt, :, tx:tx + FREE],
                                            start=(step == 0 and tx == 0),
                                            stop=(step == 2 and tx == 2),
                                            perf_mode=mybir.MatmulPerfMode.DoubleRow,
                                        )
                                pvw = pst[:].rearrange("p (b q) -> p b q",
                                                       b=GBL)[:, :, 0:W]
                                if with_bias:
                                    nc.scalar.sign(ovg[:, ji], pvw,
                                                   bias=b_sb[:, kg:kg + 1])
                                else:
                                    # DVE clamp == sign for integers; emitted
                                    # right after each 9-matmul half so the
                                    # ji=0 clamp overlaps the ji=1 matmuls.
                                    nc.vector.tensor_scalar(
                                        ovg[:, ji], pvw, 1.0, -1.0,
                                        mybir.AluOpType.min,
                                        mybir.AluOpType.max)
                            # the tail-critical final store goes out on the
                            # otherwise-idle sync queue
                            seng = nc.sync if g == NG - 1 else nc.gpsimd
                            seng.dma_start(
                                o_d[n, kg * 128:(kg + 1) * 128,
                                    g * 14:(g + 1) * 14, :],
                                osb[:, g * 784:(g + 1) * 784])
                            continue
                        ps = {}
                        # j order (1,2,0,3): m1/m2 finish first so staging
                        # starts early; m0/m3 (read by the late u-ops) are
                        # the last banks the next-next group waits on.
                        for j in (1, 2, 0, 3):
                            ps[j] = p_pool.tile([128, FREE], dt.float32,
                                                tag="ps",
                                                name=f"ps{n}_{kg}_{g}_{j}")
                            sl = kg * 4 + SLOT[j]
                            for tx in range(3):
                                nc.tensor.matmul(
                                    ps[j][:], wslot(sl, tx),
                                    cv[:, j, :, tx:tx + FREE],
                                    start=(tx == 0), stop=(tx == 2),
                                    perf_mode=mybir.MatmulPerfMode.DoubleRow,
                                )
                        # u0 = (m1+m2)+m0, u1 = (m1-m2)-m3, all staged fp16
                        # (exact, see module docstring). The last two normal
                        # groups move sm1/s0 to DVE and s1 to GpSimd so the
                        # tail chain dodges Scalar's sign backlog.
                        tailish = False
                        sm1 = upool.tile([128, FREE], dt.float16, tag="sm1")
                        sm2 = upool.tile([128, FREE], dt.float16, tag="sm2")
                        s0 = upool.tile([128, FREE], dt.float16, tag="s0")
                        s1 = upool.tile([128, FREE], dt.float16, tag="s1")
                        u01 = upool.tile([128, 2 * FREE], dt.float16, tag="u01")
                        if tailish:
                            nc.vector.tensor_scalar_add(sm1[:], ps[1][:], 0.0)
                            nc.scalar.copy(sm2[:], ps[2][:])
                            nc.vector.tensor_tensor(s0[:], sm1[:], sm2[:], add)
                            nc.gpsimd.tensor_tensor(s1[:], sm1[:], sm2[:], sub)
                        else:
                            nc.scalar.copy(sm1[:], ps[1][:])
                            nc.scalar.copy(sm2[:], ps[2][:])
                            nc.gpsimd.tensor_tensor(s0[:], sm1[:], sm2[:], add)
                            nc.vector.tensor_tensor(s1[:], sm1[:], sm2[:], sub)
                        nc.vector.tensor_tensor(u01[:, 0:FREE], s0[:], ps[0][:], add)
                        nc.vector.tensor_tensor(u01[:, FREE:], s1[:], ps[3][:], sub)
                        uv = u01[:].rearrange("p (j b q) -> p j b q", j=2,
                                              b=GBL)[:, :, :, 0:W]
                        if with_bias:
                            nc.scalar.sign(ovg, uv, bias=b_sb[:, kg:kg + 1])
                        else:
                            nc.scalar.sign(ovg, uv)
                        if g == NG - 1:
                            # one big store per (img, kg): 3136B descriptors
                            nc.gpsimd.dma_start(
                                o_d[n, kg * 128:(kg + 1) * 128, :, :],
                                osb[:, 0:3136])

    nc.finalize()
    return nc


_T_LUT = np.array([0xB8, 0xB0, 0x00, 0x30, 0x38], np.uint8)       # v/2, v=-2..2
_W_LUT = np.array([0xBC, 0xB8, 0xB0, 0x00, 0x30, 0x38, 0x3C], np.uint8)


def _prep_inputs(x, wsgn):
    """x [32,256,56,56] f32 + wsgn [128, 7680] -> per-core [8][128, TOT]."""
    s = np.sign(x).astype(np.int8)
    xp = np.zeros((N_CORES * N_PER, C, 58, 58), np.int8)
    xp[:, :, 1:57, 1:57] = s
    d0 = xp[:, :, 0:56:2, :]
    d1 = xp[:, :, 1:57:2, :]
    d2 = xp[:, :, 2:58:2, :]
    d3 = xp[:, :, 3:58:2, :]
    t = np.empty((N_CORES * N_PER, C, 4, NBL, 58), np.int8)
    t[:, :, 0] = d0 - d2
    t[:, :, 1] = d1 + d2
    t[:, :, 2] = d2 - d1
    t[:, :, 3] = d1 - d3
    tb = _T_LUT[t[..., 0:WROW] + 2]                       # [32,256,4,28,57]
    v = tb.reshape(N_CORES, N_PER, 2, 128, 4, NG, GBL, WROW)
    v = v.transpose(0, 3, 1, 5, 4, 2, 6, 7)   # [core,p,n,g,j,i,b,w]
    ch = np.zeros((N_CORES, 128, N_PER, NG, 4, 2, CW), np.uint8)
    ch[..., :FREE] = v.reshape(N_CORES, 128, N_PER, NG, 4, 2, FREE)
    ch = ch.reshape(N_CORES, 128, 16, CH)
    out = np.empty((N_CORES, 128, TOT), np.uint8)
    out[:, :, OW0:OW0 + 3072] = wsgn[:, 0:3072]
    out[:, :, OG0:OG0 + CH] = ch[:, :, 0]
    out[:, :, OW1:OW1 + 4608] = wsgn[:, 3072:7680]
    out[:, :, OT:] = ch[:, :, 1:].reshape(N_CORES, 128, 15 * CH)
    return out.view(mybir.dt.np(mybir.dt.float8e4))


def _prep_weights(weight):
    s = np.sign(weight.astype(np.float32)).astype(np.int8)  # [k, c, ty, tx]
    w0, w1, w2 = s[:, :, 0, :], s[:, :, 1, :], s[:, :, 2, :]
    g = np.empty((6, K, C, 3), np.int8)  # 2*(G w)_j (+ negated j2/j3)
    g[0] = 2 * w0
    g[1] = w0 + w1 + w2
    g[2] = w0 - w1 + w2
    g[3] = 2 * w2
    g[4] = -g[2]
    g[5] = -g[3]
    gb = _W_LUT[g + 3]
    full = gb.reshape(6, 2, 128, 2, 128, 3)      # [j, kg, kk, i, p, tx]
    order = [(1, 0), (2, 0), (0, 0), (3, 0),
             (1, 1), (2, 1), (0, 1), (3, 1), (4, 1), (5, 1)]
    sl = np.stack([full[j, kg] for (j, kg) in order])  # [s, kk, i, p, tx]
    arr = sl.transpose(3, 0, 4, 2, 1)            # [p, s, tx, i, kk]
    return np.ascontiguousarray(arr).reshape(128, 10 * 768)


def kernel(x, weight, bias, _profile=False, _trace_kwargs=None):
    x = np.asarray(x, dtype=np.float32)
    weight = np.asarray(weight, dtype=np.float32)
    bias = np.asarray(bias, dtype=np.float32)
    assert x.shape == (N_CORES * N_PER, C, H, W), x.shape
    assert weight.shape == (K, C, 3, 3), weight.shape
    assert bias.shape == (K,), bias.shape
    with_bias = bool(np.any(bias != 0.0))

    if with_bias not in _cache:
        _cache[with_bias] = _build(with_bias)
    nc = _cache[with_bias]

    wsgn = _prep_weights(weight)
    tin = _prep_inputs(x, wsgn)
    in_maps = []
    for c in range(N_CORES):
        m = {"tin": np.ascontiguousarray(tin[c])}
        if with_bias:
            m["bhalf"] = np.ascontiguousarray(
                (bias.reshape(2, 128).T * 0.5).astype(np.float32))
        in_maps.append(m)

    res = run_bass_kernel_spmd(
        nc, in_maps, core_ids=list(range(N_CORES)),
        trace=_profile, **(_trace_kwargs or {}),
    )
    out = np.concatenate([res.results[c]["out"] for c in range(N_CORES)],
                         axis=0).astype(np.float32)
    if _profile:
        kernel.last_exec_ns = res.exec_time_ns
        kernel.last_results = res
    return out


# revision 19
# speedup vs baseline: 1.0274x; 1.0001x over previous
"""Binarized 3x3 conv (XNOR-style): sign(conv2d(sign(x), sign(w)) + b).

Full-input contract: kernel(x=[32,256,56,56]f32, weight=[256,256,3,3]f32,
bias=[256]f32) -> [32,256,56,56]f32.

Strategy: data-parallel over batch across 8 NeuronCores (4 images/core),
with a 1D Winograd F(2,3) factorization along H that cuts tensor-engine
work 1.5x vs the direct 9-tap formulation (12 instead of 18 row-convs per
2 output rows).

Host prep (exact, integer-valued):
  - t_j = (B^T d)/2 over padded row quadruples d (rows 2b..2b+3 of the
    0-padded image), j=0..3: values in {0,+-0.5,+-1}, stored fp8e4m3.
    Block rows are 57 wide ([Z, r0..r55], one shared zero column), and a
    (g, j) chunk is 7 rows + 2 zero cols = 401, so the tap-tx matmul
    window [401*j + tx : .. + 399] stays inside its own chunk: free dim
    399 vs 406 (-1.7% PE) and no cross-chunk dependencies.
  - w_j = (G sign(w))_j rows: values {+-0.5,+-1,+-1.5}, exact in fp8.
    Slots in consumption order (kg0 j1,j2,j0,j3 | kg1 same | kg1
    -j2,-j3); kg0 needs no negated slots (the direct-PSUM final group is
    kg1).
  - Everything ships in ONE DRAM tensor laid out [w-kg0 | img0-group0 |
    w-kg1 | remaining 15 (img, group) chunks]: DMA cost is ~175ns per
    128-partition descriptor per queue (rows below ~3.2KB are
    latency-bound, not wire-bound) and queues process descriptors FIFO,
    so the whole startup is ordered by consumption deadline and the
    critical first transfer [w-kg0 | g0] is a single 8-descriptor-per-
    queue DMA (~2us) instead of two (~2.9us).
Device per core:
  - Boot: engines boot at ~5.8us; each dma_start costs ~700ns of
    descriptor-gen on its issuing engine, and each DGE ring (scalar /
    sync / gpsimd) delivers a 128-descriptor load in ~3.3us, so the
    critical stream {w-kg0, g0..g3, w-kg1} is spread across all three
    rings in consumption-deadline order (kg1 reuses the same t chunks,
    so after w-kg1 lands the PE has 16.6us of work queued). The PE runs
    NWARM warmup matmuls (gated only on the gpsimd wsrc memset) until
    the first chunks land ~11.6us.
    CRITICAL: the PE must stay continuously busy from the first warmup
    through the steady stream -- any idle gap during the ~3us clock-ramp
    window locks the DVFS low for the whole run (measured ~10us loss).
  - per (img, kg, group of 7 blocks): 12 fp8 DoubleRow matmuls (contract
    256, free 399) accumulate m_0..m_3 into 4 PSUM banks, emitted in j
    order (1,2,0,3) so staging starts early.
  - evac (ops may read at most one PSUM input; GpSimd none): Scalar
    stages sm1=m1/sm2=m2 to fp16 (exact: m are quarter-integers far
    below fp16's 0.25-step bound of 512, checked by the rel-err gate),
    GpSimd forms s0=sm1+sm2, DVE forms s1=sm1-sm2 plus u0=s0+m0 /
    u1=s1-m3 in fp16 (exact: u values are INTEGERS -- half of the even
    +-1 conv sum -- bounded by 1152 < 2048), Scalar applies
    Sign(u + bias/2) into the (img, kg) staging tile (Sign(0)=0 on HW).
  - Output stores are ONE [128, 3136] DMA per (img, kg) issued by GpSimd
    after its 4th group (784B-row per-group stores would cost 4x the
    descriptor time), except the final (img, kg) which stores g0-g1
    when their signs land and g2/g3 per-group so the tail transfer is
    one small [128, 784] DMA.
  - The last TWO groups accumulate u0/u1 directly in PSUM (18 matmuls
    each, using negated j2/j3 weight slots) and clamp on DVE: every
    engine's queue fully drains during those ~6us of matmuls, leaving
    only clamp(u1) + one small store behind the last matmul.
  - All sums are multiples of 0.25 bounded << 2^24 so f32 accumulation is
    exact; sign(conv+b) == sign(conv/2+b/2) by binade-shift exactness.
Output returned as fp8 (+-1/0 exact) and widened to f32 on host.
"""

import numpy as np

import concourse.bacc as bacc
import concourse.mybir as mybir
import concourse.tile as tile
from concourse.bass_utils import run_bass_kernel_spmd

N_CORES = 8
N_PER = 4          # images per core
C = 256            # input channels
K = 256            # output channels
H = W = 56
NBL = 28           # Winograd 2-row blocks per image
NG = 4             # block groups per (img, kg)
GBL = 7            # blocks per group
WROW = 57          # padded row width (1 shared zero col per block row)
FREE = GBL * WROW  # 399 matmul free size
CW = FREE + 2      # 401: (j, i) chunk col width (2 dead pad cols)
CH = 4 * 2 * CW    # 3208: per-(img, group) slab chunk
SLOT = {1: 0, 2: 1, 0: 2, 3: 3}  # j -> slot within a kg block
OW0 = 0            # w kg0 slots (4 x 768)
OG0 = 3072         # img0 group0 chunk
OW1 = OG0 + CH     # 6280: w kg1 + negated slots (6 x 768)
OT = OW1 + 4608    # 10888: remaining (img, group) chunks, deadline order
TOT = OT + 15 * CH  # 59008 columns
NWARM = 22

_cache = {}


def _build(with_bias):
    dt = mybir.dt
    xdt = dt.float8e4
    nc = bacc.Bacc()
    t_d = nc.declare_dram_parameter("tin", [128, TOT], xdt, isOutput=False)
    if with_bias:
        b_d = nc.declare_dram_parameter("bhalf", [128, 2], dt.float32,
                                        isOutput=False)
    o_d = nc.declare_dram_parameter("out", [N_PER, K, H, W], xdt, isOutput=True)

    def chunk_off(n, g):
        return OG0 if (n, g) == (0, 0) else OT + (n * NG + g - 1) * CH

    with tile.TileContext(nc) as tc:
        with (
            tc.tile_pool(name="wpool", bufs=1) as wpool,
            tc.tile_pool(name="upool", bufs=4) as upool,
            tc.tile_pool(name="opool", bufs=2) as o_pool,
            tc.tile_pool(name="psum", bufs=8, space="PSUM") as p_pool,
        ):
            wsrc = wpool.tile([128, 256], xdt)
            big = wpool.tile([128, TOT], xdt)

            # GpSimd: warmup-source memset first (gates PE warmups).
            nc.gpsimd.memset(wsrc[:], 0.0)


            # Each DGE ring (scalar-HWDGE, sync-HWDGE, gpsimd-SWDGE)
            # delivers a 128-descriptor load in ~3.3us, serially per
            # ring, so the startup stream is spread across rings in
            # consumption-deadline order. Deadlines (us): w-kg0+g0
            # ~11.6 (gates mm1), g1 ~14, g2 ~16.3, g3 ~18.4,
            # w-kg1 ~20.5, img1 ~28.8, img2 ~45, img3 ~62.
            def load(eng, lo, hi, p0=0, p1=128):
                eng.dma_start(big[p0:p1, lo:hi], t_d[p0:p1, lo:hi])

            load(nc.scalar, OG0, OG0 + CH)                 # g0
            load(nc.scalar, chunk_off(0, 3), chunk_off(1, 0))   # g3
            load(nc.scalar, OW1, OT)                       # w-kg1 + neg
            load(nc.scalar, chunk_off(1, 2), chunk_off(2, 0))   # img1 g2,g3
            load(nc.scalar, chunk_off(3, 0), TOT)              # img3
            load(nc.sync, 0, 3072)                         # w-kg0
            load(nc.sync, chunk_off(0, 2), chunk_off(0, 3))    # g2
            load(nc.sync, chunk_off(1, 0), chunk_off(1, 2))    # img1 g0,g1
            load(nc.sync, chunk_off(2, 0), chunk_off(3, 0))    # img2
            load(nc.gpsimd, chunk_off(0, 1), chunk_off(0, 2))  # g1
            if with_bias:
                nc.scalar.dma_start(b_sb := wpool.tile([128, 2], dt.float32),
                                    b_d[:])

            # Warm the PE HAM clock gate while the first chunks stream in.
            warm = p_pool.tile([128, 232], dt.float32, tag="ps")
            for _ in range(NWARM):
                nc.tensor.matmul(warm[:], wsrc[:, 0:128], wsrc[:, 0:232],
                                 start=True, stop=True)

            wv0 = big[:, OW0:OW0 + 3072].rearrange(
                "p (s t i k) -> p s t i k", s=4, t=3, i=2)
            wv1 = big[:, OW1:OW1 + 4608].rearrange(
                "p (s t i k) -> p s t i k", s=6, t=3, i=2)

            def wslot(sl, tx):
                return wv0[:, sl, tx] if sl < 4 else wv1[:, sl - 4, tx]

            add, sub = mybir.AluOpType.add, mybir.AluOpType.subtract

            for n in range(N_PER):
                for kg in range(2):
                    osb = None
                    for g in range(NG):
                        if g == 0:
                            osb = o_pool.tile([128, NG * 2 * GBL * W], xdt,
                                              tag="osb")
                        ovg = osb[:, g * 784:(g + 1) * 784].rearrange(
                            "p (b j c) -> p j b c", j=2, c=W)
                        off = chunk_off(n, g)
                        cv = big[:, off:off + CH].rearrange(
                            "p (j i c) -> p j i c", j=4, i=2)
                        # Last two groups: accumulate u0/u1 directly in PSUM
                        # via 18 matmuls each (slots 8/9 hold negated j2/j3),
                        # so both outputs come straight from clamp(psum) with
                        # no staging chain -- +2us of PE, but every engine's
                        # queue fully drains during the ~6us of direct
                        # matmuls, leaving only clamp(u1) + one small store
                        # behind the last matmul.
                        direct = (n == N_PER - 1 and kg == 1 and g >= NG - 2)
                        if direct:
                            if g == NG - 2:
                                # store g0-g1 as soon as their signs land
                                nc.gpsimd.dma_start(
                                    o_d[n, kg * 128:(kg + 1) * 128, 0:28, :],
                                    osb[:, 0:1568])
                            for ji, jws in ((0, ((4, 1), (5, 2), (6, 0))),
                                            (1, ((4, 1), (8, 2), (9, 3)))):
                                pst = p_pool.tile([128, FREE], dt.float32,
                                                  tag="ps", name=f"psF{g}_{ji}")
                                for step, (sl, jt) in enumerate(jws):
                                    for tx in range(3):
                                        nc.tensor.matmul(
                                            pst[:], wslot(sl, tx),
                                            cv[:, jt, :, tx:tx + FREE],
                                            start=(step == 0 and tx == 0),
                                            stop=(step == 2 and tx == 2),
                                            perf_mode=mybir.MatmulPerfMode.DoubleRow,
                                        )
                                pvw = pst[:].rearrange("p (b q) -> p b q",
                                                       b=GBL)[:, :, 0:W]
                                if with_bias:
                                    nc.scalar.sign(ovg[:, ji], pvw,
                                                   bias=b_sb[:, kg:kg + 1])
                                else:
                                    # DVE clamp == sign for integers; emitted
                                    # right after each 9-matmul half so the
                                    # ji=0 clamp overlaps the ji=1 matmuls.
                                    nc.vector.tensor_scalar(
                                        ovg[:, ji], pvw, 1.0, -1.0,
                                        mybir.AluOpType.min,
                                        mybir.AluOpType.max)
                            # the tail-critical final store goes out on the
                            # otherwise-idle sync queue
                            seng = nc.sync if g == NG - 1 else nc.gpsimd
                            seng.dma_start(
                                o_d[n, kg * 128:(kg + 1) * 128,
                                    g * 14:(g + 1) * 14, :],
                                osb[:, g * 784:(g + 1) * 784])
                            continue
                        ps = {}
                        # j order (1,2,0,3): m1/m2 finish first so staging
                        # starts early; m0/m3 (read by the late u-ops) are
                        # the last banks the next-next group waits on.
                        for j in (1, 2, 0, 3):
                            ps[j] = p_pool.tile([128, FREE], dt.float32,
                                                tag="ps",
                                                name=f"ps{n}_{kg}_{g}_{j}")
                            sl = kg * 4 + SLOT[j]
                            for tx in range(3):
                                nc.tensor.matmul(
                                    ps[j][:], wslot(sl, tx),
                                    cv[:, j, :, tx:tx + FREE],
                                    start=(tx == 0), stop=(tx == 2),
                                    perf_mode=mybir.MatmulPerfMode.DoubleRow,
                                )
                        # u0 = (m1+m2)+m0, u1 = (m1-m2)-m3, all staged fp16
                        # (exact, see module docstring). The last two normal
                        # groups move sm1/s0 to DVE and s1 to GpSimd so the
                        # tail chain dodges Scalar's sign backlog.
                        tailish = False
                        sm1 = upool.tile([128, FREE], dt.float16, tag="sm1")
                        sm2 = upool.tile([128, FREE], dt.float16, tag="sm2")
                        s0 = upool.tile([128, FREE], dt.float16, tag="s0")
                        s1 = upool.tile([128, FREE], dt.float16, tag="s1")
                        u01 = upool.tile([128, 2 * FREE], dt.float16, tag="u01")
                        if tailish:
                            nc.vector.tensor_scalar_add(sm1[:], ps[1][:], 0.0)
                            nc.scalar.copy(sm2[:], ps[2][:])
                            nc.vector.tensor_tensor(s0[:], sm1[:], sm2[:], add)
                            nc.gpsimd.tensor_tensor(s1[:], sm1[:], sm2[:], sub)
                        else:
                            nc.scalar.copy(sm1[:], ps[1][:])
                            nc.scalar.copy(sm2[:], ps[2][:])
                            nc.gpsimd.tensor_tensor(s0[:], sm1[:], sm2[:], add)
                            nc.vector.tensor_tensor(s1[:], sm1[:], sm2[:], sub)
                        nc.vector.tensor_tensor(u01[:, 0:FREE], s0[:], ps[0][:], add)
                        nc.vector.tensor_tensor(u01[:, FREE:], s1[:], ps[3][:], sub)
                        uv = u01[:].rearrange("p (j b q) -> p j b q", j=2,
                                              b=GBL)[:, :, :, 0:W]
                        if with_bias:
                            nc.scalar.sign(ovg, uv, bias=b_sb[:, kg:kg + 1])
                        else:
                            nc.scalar.sign(ovg, uv)
                        if g == NG - 1:
                            # one big store per (img, kg): 3136B descriptors
                            nc.gpsimd.dma_start(
                                o_d[n, kg * 128:(kg + 1) * 128, :, :],
                                osb[:, 0:3136])

    nc.finalize()
    return nc


_T_LUT = np.array([0xB8, 0xB0, 0x00, 0x30, 0x38], np.uint8)       # v/2, v=-2..2
_W_LUT = np.array([0xBC, 0xB8, 0xB0, 0x00, 0x30, 0x38, 0x3C], np.uint8)


def _prep_inputs(x, wsgn):
    """x [32,256,56,56] f32 + wsgn [128, 7680] -> per-core [8][128, TOT]."""
    s = np.sign(x).astype(np.int8)
    xp = np.zeros((N_CORES * N_PER, C, 58, 58), np.int8)
    xp[:, :, 1:57, 1:57] = s
    d0 = xp[:, :, 0:56:2, :]
    d1 = xp[:, :, 1:57:2, :]
    d2 = xp[:, :, 2:58:2, :]
    d3 = xp[:, :, 3:58:2, :]
    t = np.empty((N_CORES * N_PER, C, 4, NBL, 58), np.int8)
    t[:, :, 0] = d0 - d2
    t[:, :, 1] = d1 + d2
    t[:, :, 2] = d2 - d1
    t[:, :, 3] = d1 - d3
    tb = _T_LUT[t[..., 0:WROW] + 2]                       # [32,256,4,28,57]
    v = tb.reshape(N_CORES, N_PER, 2, 128, 4, NG, GBL, WROW)
    v = v.transpose(0, 3, 1, 5, 4, 2, 6, 7)   # [core,p,n,g,j,i,b,w]
    ch = np.zeros((N_CORES, 128, N_PER, NG, 4, 2, CW), np.uint8)
    ch[..., :FREE] = v.reshape(N_CORES, 128, N_PER, NG, 4, 2, FREE)
    ch = ch.reshape(N_CORES, 128, 16, CH)
    out = np.empty((N_CORES, 128, TOT), np.uint8)
    out[:, :, OW0:OW0 + 3072] = wsgn[:, 0:3072]
    out[:, :, OG0:OG0 + CH] = ch[:, :, 0]
    out[:, :, OW1:OW1 + 4608] = wsgn[:, 3072:7680]
    out[:, :, OT:] = ch[:, :, 1:].reshape(N_CORES, 128, 15 * CH)
    return out.view(mybir.dt.np(mybir.dt.float8e4))


def _prep_weights(weight):
    s = np.sign(weight.astype(np.float32)).astype(np.int8)  # [k, c, ty, tx]
    w0, w1, w2 = s[:, :, 0, :], s[:, :, 1, :], s[:, :, 2, :]
    g = np.empty((6, K, C, 3), np.int8)  # 2*(G w)_j (+ negated j2/j3)
    g[0] = 2 * w0
    g[1] = w0 + w1 + w2
    g[2] = w0 - w1 + w2
    g[3] = 2 * w2
    g[4] = -g[2]
    g[5] = -g[3]
    gb = _W_LUT[g + 3]
    full = gb.reshape(6, 2, 128, 2, 128, 3)      # [j, kg, kk, i, p, tx]
    order = [(1, 0), (2, 0), (0, 0), (3, 0),
             (1, 1), (2, 1), (0, 1), (3, 1), (4, 1), (5, 1)]
    sl = np.stack([full[j, kg] for (j, kg) in order])  # [s, kk, i, p, tx]
    arr = sl.transpose(3, 0, 4, 2, 1)            # [p, s, tx, i, kk]
    return np.ascontiguousarray(arr).reshape(128, 10 * 768)


def kernel(x, weight, bias, _profile=False, _trace_kwargs=None):
    x = np.asarray(x, dtype=np.float32)
    weight = np.asarray(weight, dtype=np.float32)
    bias = np.asarray(bias, dtype=np.float32)
    assert x.shape == (N_CORES * N_PER, C, H, W), x.shape
    assert weight.shape == (K, C, 3, 3), weight.shape
    assert bias.shape == (K,), bias.shape
    with_bias = bool(np.any(bias != 0.0))

    if with_bias not in _cache:
        _cache[with_bias] = _build(with_bias)
    nc = _cache[with_bias]

    wsgn = _prep_weights(weight)
    tin = _prep_inputs(x, wsgn)
    in_maps = []
    for c in range(N_CORES):
        m = {"tin": np.ascontiguousarray(tin[c])}
        if with_bias:
            m["bhalf"] = np.ascontiguousarray(
                (bias.reshape(2, 128).T * 0.5).astype(np.float32))
        in_maps.append(m)

    res = run_bass_kernel_spmd(
        nc, in_maps, core_ids=list(range(N_CORES)),
        trace=_profile, **(_trace_kwargs or {}),
    )
    out = np.concatenate([res.results[c]["out"] for c in range(N_CORES)],
                         axis=0).astype(np.float32)
    if _profile:
        kernel.last_exec_ns = res.exec_time_ns
        kernel.last_results = res
    return out
